# revision 15
# baseline (speedup 1.0000x reference)
"""Deep Lagrangian Network forward dynamics on 8 Trainium2 NeuronCores.

Self-contained Bass/Tile kernel. Data-parallel: batch 65536 split as 8192
samples per core; MLP weights replicated.

Math restructure (vs the reference's jacfwd over 7 tangents):
  H_dq.qd == H_dt = L Ldt^T + Ldt L^T (symmetric)  =>  c = 0.5 * H_dt @ qd.
  Only the directional derivative of the net along qd is needed, so the
  kernel runs ONE tangent stream:
    tz_l = th_{l-1} @ W_l,   th_l = sigmoid(z_l) * tz_l
  Activations use a single ACT table set (natural_log_exp_and_others):
    E = exp(z); h = softplus(z) = ln(E+1); sigmoid scaling via
    E2 = exp(-h) = 1/(1+e^z):  th = (E2 - 1) * tz  (one fused DVE op,
    sign fixed by seeding the tangent with -W0: (-1)^3 * (-1) = +1).

Per-sample 7x7 algebra (L build, H = L L^T, tau/c) is done batched with
features on partitions and samples on the free dim, using constant 0/1
"selection" matmuls on the tensor engine to duplicate/reduce rows:
    u = L^T qdd, tau1 = L u, b = L^T qd, a = Ldt^T qd,
    tau = tau1 + 0.5(L a + Ldt b) + g   (all folded into masked matmuls)

Output is packed as one [B, 96] tensor (tau|c|g|pad|H49|pad) transposed
on-chip via the DVE 32x32 block transpose + a swizzled output DMA; the
host slices the real columns.
"""

import os
import sys

import numpy as np

for _p in ("/opt/trn_rl_repo", "/root/.axon_site/_ro/trn_rl_repo"):
    if os.path.isdir(_p) and _p not in sys.path:
        sys.path.insert(0, _p)
        break

import concourse.bass as bass
import concourse.mybir as mybir
import concourse.tile as tile
from concourse import bacc

F32 = mybir.dt.float32
AF = mybir.ActivationFunctionType
ALU = mybir.AluOpType

N = 7
WIDTH = 256
N_TRIL = 21
OFFSET = 1e-3
N_CORES = 8
BATCH = 65536
PER = BATCH // N_CORES  # 8192
GB = 512  # samples per group (psum bank free-dim)
OUTC = 96  # padded output columns per sample

# ---------------------------------------------------------------- enumerations
_rows, _cols = np.tril_indices(N, -1)
PAIRS = [(i, i) for i in range(N)] + list(zip(_rows.tolist(), _cols.tolist()))
PIDX = {p: t for t, p in enumerate(PAIRS)}  # 28 pairs: diag first, tril after
TRIPLES = [(i, k, j) for i in range(N) for k in range(i + 1) for j in range(k + 1)]
assert len(PAIRS) == 28 and len(TRIPLES) == 84

# mask image column offsets (RH padded to 64 cols, R2 to 32 cols so the
# reduce matmuls also zero-fill the psum pad rows)
_MA, _MB, _RH, _ML1, _ML2, _MQ1, _MQ2, _R1, _MU, _R2 = (
    0, 84, 168, 232, 296, 324, 416, 508, 529, 621)
MKCOLS = 653
CSTCOLS = 1706 + MKCOLS + 8  # one packed constant image -> a single DMA


def _build_masks():
    # S/EH row space is padded: diag pairs at rows 0-6, tril pairs at rows
    # 32-52 (SBUF APs must start at partition 0/32/64/96).
    def rp(p):
        return p if p < 7 else p + 25

    mk = np.zeros((128, MKCOLS), np.float32)
    for t, (i, k, j) in enumerate(TRIPLES):
        mk[rp(PIDX[(i, j)]), _MA + t] = 1.0  # A_H row select
        mk[rp(PIDX[(k, j)]), _MB + t] = 1.0  # B_H row select
        mk[t, _RH + 7 * i + k] = 1.0         # reduce -> H[i,k]
        if i != k:
            mk[t, _RH + 7 * k + i] = 1.0
    for p in range(28):
        i, j = PAIRS[p]
        mk[rp(p), _ML1 + p] = 1.0        # LLL rows 0-27  = L28
        mk[rp(p), _ML1 + 32 + p] = 1.0   # LLL rows 32-59 = L28
        mk[rp(p), _ML2 + p] = 1.0        # LLL rows 64-91 = Ldt28 (base 64)
        mk[i, _MQ1 + p] = 1.0            # QQQ rows 0-27  = qdd[i_p]
        mk[i, _MQ2 + 32 + p] = 1.0       # QQQ rows 32-59 = qd[i_p]
        mk[i, _MQ2 + 64 + p] = 1.0       # QQQ rows 64-91 = qd[i_p]
        mk[p, _R1 + j] = 1.0             # PROD1 rows 0-27  -> u_j
        mk[32 + p, _R1 + 7 + j] = 1.0    # rows 32-59 -> b_j
        mk[64 + p, _R1 + 14 + j] = 1.0   # rows 64-91 -> a_j
        mk[j, _MU + p] = 1.0             # UAB2 rows 0-27  = u[j_p] (uba rows 0-6)
        mk[14 + j, _MU + 32 + p] = 1.0   # rows 32-59 = a[j_p] (uba rows 14-20)
        mk[7 + j, _MU + 64 + p] = 1.0    # rows 64-91 = b[j_p] (uba rows 7-13)
        mk[p, _R2 + i] = 1.0             # PROD2 rows 0-27 -> tau1_i
        mk[32 + p, _R2 + i] = 0.5        # + 0.5*ca_i into tau col
        mk[64 + p, _R2 + i] = 0.5        # + 0.5*cb_i into tau col
        mk[32 + p, _R2 + 7 + i] = 0.5    # c_i
        mk[64 + p, _R2 + 7 + i] = 0.5
    return mk


def _host_images(params):
    p = {k: np.asarray(v, np.float32) for k, v in params.items()}
    w0i = np.zeros((8, 512), np.float32)
    w0i[:7, 0:256] = p["W0"]
    w0i[7, 0:256] = p["b0"]
    w0i[:7, 256:512] = -p["W0"]  # tangent seed: negated (sign restored by 3 stt ops)
    w12 = np.zeros((128, 1024), np.float32)
    w12[:, 0:256] = p["W1"][0:128]
    w12[:, 256:512] = p["W1"][128:256]
    w12[:, 512:768] = p["W2"][0:128]
    w12[:, 768:1024] = p["W2"][128:256]
    # heads lhsT, padded to M=53: cols 0-6 = Wd, 32-52 = Wt (out rows match S)
    uh = np.zeros((128, 106), np.float32)
    for kc in range(2):
        uh[:, 53 * kc + 0:53 * kc + 7] = p["Wd"][128 * kc:128 * (kc + 1)]
        uh[:, 53 * kc + 32:53 * kc + 53] = p["Wt"][128 * kc:128 * (kc + 1)]
    ga = np.zeros((128, 64), np.float32)  # M=32 per K-chunk (rows 21-31 zero)
    for kc in range(2):
        ga[:, 32 * kc + 0:32 * kc + 7] = p["Wg"][128 * kc:128 * (kc + 1)]
        ga[:, 32 * kc + 14:32 * kc + 21] = p["Wg"][128 * kc:128 * (kc + 1)]
    cv = np.zeros((128, 8), np.float32)
    cv[:, 0] = p["b1"][0:128]
    cv[:, 1] = p["b1"][128:256]
    cv[:, 2] = p["b2"][0:128]
    cv[:, 3] = p["b2"][128:256]
    cv[0:7, 4] = p["bd"] + np.float32(OFFSET)  # Ld = max(zd+bd+off, off)
    cv[32:53, 4] = p["bt"]                     # Lt = zt + bt
    cv[0:7, 5] = p["bd"]                       # mask = (zd+bd) > 0
    cv[0:7, 6] = p["bg"]                       # tau += g + bg
    cv[14:21, 6] = p["bg"]                     # g out row
    mk = _build_masks()
    cst = np.zeros((128, CSTCOLS), np.float32)
    cst[0:8, 0:512] = w0i
    cst[:, 512:1536] = w12
    cst[:, 1536:1642] = uh
    cst[:, 1642:1706] = ga
    cst[:, 1706:1706 + MKCOLS] = mk
    cst[:, 1706 + MKCOLS:1706 + MKCOLS + 8] = cv
    return {"cst": cst}


def _emit_program(per_core):
    """Build the single-core Bass program (SPMD: same NEFF on all cores)."""
    ng = per_core // GB
    nc = bacc.Bacc("TRN2", target_bir_lowering=False, debug=False)
    d = {}
    d["xin"] = nc.declare_dram_parameter("xin", [8, 3 * per_core], F32, isOutput=False).ap()
    d["cst"] = nc.declare_dram_parameter("cst", [128, CSTCOLS], F32, isOutput=False).ap()
    d_o = nc.declare_dram_parameter("o", [per_core * OUTC], F32, isOutput=True).ap()

    xview = d["xin"].rearrange("p (n t s) -> n p (t s)", n=ng, t=3, s=GB)
    # DRAM addressing for the block-transposed output tile: t96[32A+r, 32B+c]
    # holds out[g*512 + 32B + r, 32A + c] of the logical [per_core, 96] output.
    # One DMA per A-block keeps each AP at <=3 dims.
    oview = d_o.rearrange("(n B r A c) -> n A r B c",
                          n=ng, B=GB // 32, r=32, A=OUTC // 32, c=32)
    nblk = OUTC // 32

    with tile.TileContext(nc) as tc:
        with tc.tile_pool(name="const", bufs=1) as cp, \
             tc.tile_pool(name="sb", bufs=2) as sb, \
             tc.tile_pool(name="ps", bufs=2, space="PSUM") as ps:
            cst = cp.tile([128, CSTCOLS], F32)
            nc.sync.dma_start(out=cst, in_=d["cst"])
            w0i = cst[:, 0:512]
            w12 = cst[:, 512:1536]
            uh = cst[:, 1536:1642]
            ga = cst[:, 1642:1706]
            mk = cst[:, 1706:1706 + MKCOLS]
            cv = cst[:, 1706 + MKCOLS:1706 + MKCOLS + 8]

            for g in range(ng):
                in8 = sb.tile([8, 1536], F32, tag="in8", bufs=2, name=f"in8_{g}")
                nc.sync.dma_start(out=in8, in_=xview[g])
                hf = in8[0:8, 0:512]     # fwd stream rhs (K=8: q + ones row)
                ht = in8[0:8, 512:1024]  # tangent stream rhs (K=8: qd + zeros)
                for li in range(3):
                    zf = ps.tile([128, 1024], F32, tag="z", bufs=2, name=f"zf{li}_{g}")
                    tz = ps.tile([128, 1024], F32, tag="z", bufs=2, name=f"tz{li}_{g}")
                    if li == 0:
                        for c in range(2):
                            nc.tensor.matmul(zf[:, c * 512:(c + 1) * 512],
                                             w0i[0:8, c * 128:(c + 1) * 128], hf,
                                             start=True, stop=True)
                            nc.tensor.matmul(tz[:, c * 512:(c + 1) * 512],
                                             w0i[0:8, 256 + c * 128:256 + (c + 1) * 128],
                                             ht, start=True, stop=True)
                    else:
                        wofs = (li - 1) * 512
                        for c in range(2):
                            for kc in range(2):
                                lh = w12[:, wofs + kc * 256 + c * 128:
                                         wofs + kc * 256 + (c + 1) * 128]
                                nc.tensor.matmul(zf[:, c * 512:(c + 1) * 512], lh,
                                                 hf[:, kc * 512:(kc + 1) * 512],
                                                 start=(kc == 0), stop=(kc == 1))
                            for kc in range(2):
                                lh = w12[:, wofs + kc * 256 + c * 128:
                                         wofs + kc * 256 + (c + 1) * 128]
                                nc.tensor.matmul(tz[:, c * 512:(c + 1) * 512], lh,
                                                 ht[:, kc * 512:(kc + 1) * 512],
                                                 start=(kc == 0), stop=(kc == 1))
                    E = sb.tile([128, 1024], F32, tag="E", bufs=2, name=f"E{li}_{g}")
                    if li == 0:
                        nc.scalar.activation(E, zf, AF.Exp)  # bias is in the matmul
                    else:
                        for c in range(2):
                            nc.scalar.activation(E[:, c * 512:(c + 1) * 512],
                                                 zf[:, c * 512:(c + 1) * 512], AF.Exp,
                                                 bias=cv[:, 2 * (li - 1) + c:
                                                         2 * (li - 1) + c + 1])
                    hn = sb.tile([128, 1024], F32, tag="h", bufs=4, name=f"h{li}_{g}")
                    nc.scalar.activation(hn, E, AF.Ln, bias=1.0)
                    E2 = sb.tile([128, 1024], F32, tag="E2", bufs=2, name=f"E2{li}_{g}")
                    nc.scalar.activation(E2, hn, AF.Exp, scale=-1.0)
                    thn = sb.tile([128, 1024], F32, tag="th", bufs=3, name=f"th{li}_{g}")
                    nc.vector.scalar_tensor_tensor(thn, E2, 1.0, tz,
                                                   ALU.subtract, ALU.mult)
                    hf, ht = hn, thn
                h2, th2 = hf, ht

                ehf = ps.tile([53, 512], F32, tag="fin", bufs=4, name=f"ehf_{g}")
                eht = ps.tile([53, 512], F32, tag="fin", bufs=4, name=f"eht_{g}")
                for kc in range(2):
                    nc.tensor.matmul(ehf, uh[:, kc * 53:(kc + 1) * 53],
                                     h2[:, kc * 512:(kc + 1) * 512],
                                     start=(kc == 0), stop=(kc == 1))
                for kc in range(2):
                    nc.tensor.matmul(eht, uh[:, kc * 53:(kc + 1) * 53],
                                     th2[:, kc * 512:(kc + 1) * 512],
                                     start=(kc == 0), stop=(kc == 1))

                S = sb.tile([53, 1024], F32, tag="S", bufs=2, name=f"S_{g}")
                nc.gpsimd.memset(S, 0.0)  # zero the pad rows 7-31 (gpsimd is idle)
                msk = sb.tile([7, 512], F32, tag="msk", bufs=2, name=f"msk_{g}")
                # L diag: max(zd + (bd+off), off); L tril: zt + bt
                nc.vector.tensor_scalar(S[0:7, 0:512], ehf[0:7, :], cv[0:7, 4:5],
                                        OFFSET, ALU.add, ALU.max)
                nc.vector.tensor_scalar_add(S[32:53, 0:512], ehf[32:53, :],
                                            cv[32:53, 4:5])
                nc.vector.tensor_scalar(msk, ehf[0:7, :], cv[0:7, 5:6],
                                        0.0, ALU.add, ALU.is_gt)
                nc.vector.tensor_tensor(S[0:7, 512:1024], msk, eht[0:7, :], ALU.mult)
                nc.scalar.copy(S[32:53, 512:1024], eht[32:53, :])

                pah = ps.tile([84, 512], F32, tag="fin", bufs=4, name=f"pah_{g}")
                nc.tensor.matmul(pah, mk[0:53, _MA:_MA + 84], S[0:53, 0:512],
                                 start=True, stop=True)
                pbh = ps.tile([84, 512], F32, tag="fin", bufs=4, name=f"pbh_{g}")
                nc.tensor.matmul(pbh, mk[0:53, _MB:_MB + 84], S[0:53, 0:512],
                                 start=True, stop=True)
                pahs = sb.tile([84, 512], F32, tag="prod", bufs=3, name=f"pahs_{g}")
                nc.scalar.copy(pahs, pah)  # walrus: tt reads at most one PSUM input
                phs = sb.tile([84, 512], F32, tag="prod", bufs=3, name=f"phs_{g}")
                nc.vector.tensor_tensor(phs, pahs, pbh, ALU.mult)
                hps = ps.tile([96, 512], F32, tag="fin", bufs=4, name=f"hps_{g}")
                nc.tensor.matmul(hps[32:64, :], mk[0:84, _RH:_RH + 32], phs,
                                 start=True, stop=True, skip_group_check=True)
                nc.tensor.matmul(hps[64:96, :], mk[0:84, _RH + 32:_RH + 64], phs,
                                 start=True, stop=True, skip_group_check=True)

                lll = ps.tile([92, 512], F32, tag="fin", bufs=4, name=f"lll_{g}")
                nc.tensor.matmul(lll[0:64, :], mk[0:53, _ML1:_ML1 + 64],
                                 S[0:53, 0:512], start=True, stop=True,
                                 skip_group_check=True)
                nc.tensor.matmul(lll[64:92, :], mk[0:53, _ML2:_ML2 + 28],
                                 S[0:53, 512:1024], start=True, stop=True,
                                 skip_group_check=True)
                qqq = ps.tile([92, 512], F32, tag="fin", bufs=4, name=f"qqq_{g}")
                nc.tensor.matmul(qqq, mk[0:8, _MQ1:_MQ1 + 92],
                                 in8[0:8, 1024:1536], start=True, stop=False)
                nc.tensor.matmul(qqq, mk[0:8, _MQ2:_MQ2 + 92],
                                 in8[0:8, 512:1024], start=False, stop=True)
                llls = sb.tile([92, 512], F32, tag="prod", bufs=3, name=f"llls_{g}")
                nc.scalar.copy(llls, lll)  # used twice; frees the psum slot early
                p1 = sb.tile([92, 512], F32, tag="prod", bufs=3, name=f"p1_{g}")
                nc.vector.tensor_tensor(p1, llls, qqq, ALU.mult)
                uba = ps.tile([21, 512], F32, tag="fin", bufs=4, name=f"uba_{g}")
                nc.tensor.matmul(uba, mk[0:92, _R1:_R1 + 21], p1,
                                 start=True, stop=True)
                ubas = sb.tile([21, 512], F32, tag="ubas", bufs=2, name=f"ubas_{g}")
                nc.vector.tensor_copy(ubas, uba)
                ua2 = ps.tile([92, 512], F32, tag="fin", bufs=4, name=f"ua2_{g}")
                nc.tensor.matmul(ua2, mk[0:21, _MU:_MU + 92], ubas,
                                 start=True, stop=True)
                p2 = sb.tile([92, 512], F32, tag="prod", bufs=3, name=f"p2_{g}")
                nc.vector.tensor_tensor(p2, llls, ua2, ALU.mult)

                c21 = ps.tile([32, 512], F32, tag="fin", bufs=4, name=f"c21_{g}")
                nc.tensor.matmul(c21, ga[:, 0:32], h2[:, 0:512],
                                 start=True, stop=False)
                nc.tensor.matmul(c21, ga[:, 32:64], h2[:, 512:1024],
                                 start=False, stop=False)
                nc.tensor.matmul(c21, mk[0:92, _R2:_R2 + 32], p2,
                                 start=False, stop=True)

                o96 = sb.tile([96, 512], F32, tag="o96", bufs=2, name=f"o96_{g}")
                nc.scalar.activation(o96[0:32, :], c21, AF.Identity,
                                     bias=cv[0:32, 6:7])
                nc.scalar.activation(o96[32:64, :], hps[32:64, :], AF.Identity)
                nc.scalar.activation(o96[64:96, :], hps[64:96, :], AF.Identity)
                t96 = sb.tile([96, 512], F32, tag="t96", bufs=2, name=f"t96_{g}")
                nc.vector.transpose(t96, o96)
                for A in range(nblk):
                    blk = t96[32 * A:32 * (A + 1), :].rearrange(
                        "r (B c) -> r B c", B=GB // 32, c=32)
                    nc.sync.dma_start(out=oview[g][A], in_=blk)
    nc.compile()
    return nc


def _host_xin(q, qd, qdd):
    """[8, 3*B] image, group-major: per group g the columns are
    [q^T | qd^T | qdd^T] blocks of GB; row 7 = ones for q (bias), zeros else."""
    b = q.shape[0]
    ng = b // GB
    x = np.zeros((8, ng, 3, GB), np.float32)
    for g in range(ng):
        sl = slice(g * GB, (g + 1) * GB)
        x[0:7, g, 0, :] = q[sl].T
        x[0:7, g, 1, :] = qd[sl].T
        x[0:7, g, 2, :] = qdd[sl].T
        x[7, g, 0, :] = 1.0
    return x.reshape(8, 3 * b)


_PROGRAM_CACHE = {}


def _get_program(per_core):
    if per_core not in _PROGRAM_CACHE:
        _PROGRAM_CACHE[per_core] = _emit_program(per_core)
    return _PROGRAM_CACHE[per_core]


def _unpack(o):
    o = o.reshape(-1, OUTC)
    tau = np.ascontiguousarray(o[:, 0:7])
    c = np.ascontiguousarray(o[:, 7:14])
    g = np.ascontiguousarray(o[:, 14:21])
    H = np.ascontiguousarray(o[:, 32:81]).reshape(-1, 7, 7)
    return tau, H, c, g


def _run(q, qd, qdd, params, trace=False, **spmd_kwargs):
    from concourse.bass_utils import run_bass_kernel_spmd

    q = np.asarray(q, np.float32)
    qd = np.asarray(qd, np.float32)
    qdd = np.asarray(qdd, np.float32)
    imgs = _host_images(params)
    nc = _get_program(PER)
    in_maps = []
    for cidx in range(N_CORES):
        sl = slice(cidx * PER, (cidx + 1) * PER)
        m = dict(imgs)
        m["xin"] = _host_xin(q[sl], qd[sl], qdd[sl])
        in_maps.append(m)
    res = run_bass_kernel_spmd(nc, in_maps, list(range(N_CORES)),
                               trace=trace, **spmd_kwargs)
    O = np.concatenate([res.results[i]["o"] for i in range(N_CORES)], axis=0)
    return _unpack(O), res


def kernel(q, qd, qdd, params):
    (tau, H, c, g), _ = _run(q, qd, qdd, params)
    return tau, H, c, g
